# revision 21
# baseline (speedup 1.0000x reference)
"""Trainium2 Bass kernel for nn_BaseEBM (EBM inner gradient-descent loop).

Computation (per sample, matching the reference):
    y = y_mean
    repeat 20x:  y <- y - 0.1 * dE/dy
    E(x, y) = W3 @ relu(W2 @ relu(W1 @ relu(W0 @ [x, y] + b0) + b1) + b2) + b3

Distribution: pure data parallel over 8 NeuronCores (131072 samples each).

Device layout: feature-major [128, 512] tiles. Four independent sample
groups of 32 features are block-diagonally packed across the 128
partitions; 512 samples per group per tile -> 2048 samples/tile, 64
tiles/core. All matmuls use block-diagonal weights so one instruction
processes 4 groups at full PE rate (fp32r, 1 cycle/column).

Key algebraic restructurings:
  * The energy head (layer 3 forward) is never computed; W3 is folded into
    the first backward matmul: g1p = blkdiag(W2 * W3^T) @ m2.
  * x never changes across steps, so z0 = W0 @ [x, y] is kept resident in
    PSUM for all 20 steps and updated by accumulating matmuls:
        dz0 = -lr * w0y (w0y . g0) = blkdiag(P) @ g0,  P = -lr w0y w0y^T.
  * y is never materialized during the loop.  Since dz0 = w0y * dy, the
    final y is recovered from the PSUM residue:
        y = (z0_fin[f*] - z0_init[f*]) / w0y[f*] + y_mean
    (f* = argmax |w0y|; the init snapshot cancels the init rounding).
    This frees a PSUM bank per chain, allowing FOUR independent
    tile-chains in the 8 banks - needed because one chain's per-step
    dependency path (~7 us) is ~4x its per-engine work (~1.6 us).
  * Masked backprop uses the fused DVE op (h > 0) * g in one instruction
    (scalar_tensor_tensor with is_gt + mult), so relu masks are never
    materialized for layers 0/1.
  * The transient-PSUM pool is bufs=1 per chain: the slot-reuse deps
    exactly coincide with the data deps (z2 can only start after h1,
    which is when z1's bank frees), so one bank per chain costs nothing.

Per step per tile: 5 matmuls, 2 ACT relus, 2 fused DVE mask-multiplies;
the layer-2 mask alternates between (ACT relu + GpSimd is_gt) and (DVE
is_gt on PSUM) to balance ACT vs DVE load (~60/40).
"""

import numpy as np

import concourse.bass as bass
import concourse.mybir as mybir
import concourse.tile as tile
from concourse import bacc
from concourse.bass_utils import run_bass_kernel_spmd

F32 = mybir.dt.float32
F32R = mybir.dt.float32r
ALU = mybir.AluOpType
AF = mybir.ActivationFunctionType

B = 1048576
NCORES = 8
BC = B // NCORES           # 131072 samples per core
G = 4                      # sample groups packed across partitions
TILE_N = 512               # samples per group per tile (PSUM bank limit)
SPT = G * TILE_N           # 2048 samples per tile
NT_FULL = BC // SPT        # 64 tiles per core
STEPS = 20
LR = 0.1
W = 32
NCHAINS = 4
SHARED_TMP = False
TMP_BUFS = 3
DYN = True     # hardware For_i loop over tile-quads



def _emit_tile_chain(nc, t, c, dram, wt, sb, ptmp, pz0, io, fstar, inv, cfac):
    """Generator emitting one packed tile's program; yields between steps
    so NCHAINS chains interleave in emission (and thus in the static
    per-engine schedules)."""
    _dyn = not isinstance(t, int)
    src = dram["inp0"][bass.ds(t, 1)][0, c] if _dyn else dram["inp0"][t][c]
    dst = dram["yout"][bass.ds(t, 1)][0, c] if _dyn else dram["yout"][t][c]
    inp = io.tile([2 * G, TILE_N], F32R, tag=f"inp{c}", name=f"inp_{c}")
    nc.sync.dma_start(out=inp[:], in_=src)
    xt = io.tile([G, TILE_N], F32, tag=f"xt{c}", name=f"xt_{c}")
    nc.sync.dma_start(out=xt[:], in_=src[0::2, :].bitcast(F32))

    z0 = pz0.tile([128, TILE_N], F32, tag="z0", name=f"z0_{c}")
    # z0 = blkdiag(W0) @ [x; y_mean]   (no bias; ACT adds b0 every step)
    nc.tensor.matmul(
        z0[:], wt["L0"][:], inp[:],
        start=True, stop=False, skip_group_check=True,
    )
    yield

    for s in range(STEPS):
        h0 = sb.tile([128, TILE_N], F32R, tag="h0", name=f"h0_{c}")
        nc.scalar.activation(h0[:], z0[:], AF.Relu, bias=wt["b0"][:])
        yield
        z1 = ptmp.tile([128, TILE_N], F32, tag="tmp", name=f"z1_{c}")
        nc.tensor.matmul(
            z1[:], wt["Lz1"][:], h0[:],
            start=True, stop=True, skip_group_check=True,
        )
        yield
        h1 = sb.tile([128, TILE_N], F32R, tag="h1", name=f"h1_{c}")
        nc.scalar.activation(h1[:], z1[:], AF.Relu, bias=wt["b1"][:])
        yield
        z2 = ptmp.tile([128, TILE_N], F32, tag="tmp", name=f"z2_{c}")
        nc.tensor.matmul(
            z2[:], wt["Lz2"][:], h1[:],
            start=True, stop=True, skip_group_check=True,
        )
        yield
        m2 = sb.tile([128, TILE_N], F32R, tag="m2", name=f"m2_{c}")
        # GpSimd tensor_scalar measured ~8us/op on HW - never use it.
        # ACT computes h2 (balances ACT vs DVE), DVE derives the 0/1 mask
        # from SBUF at 2-port rate.
        h2 = sb.tile([128, TILE_N], F32R, tag="h2", name=f"h2_{c}")
        nc.scalar.activation(h2[:], z2[:], AF.Relu, bias=wt["b2"][:])
        yield
        nc.vector.tensor_scalar(m2[:], h2[:], 0.0, None, ALU.is_gt)
        yield
        g1p = ptmp.tile([128, TILE_N], F32, tag="tmp", name=f"g1p_{c}")
        nc.tensor.matmul(
            g1p[:], wt["Lg1"][:], m2[:],
            start=True, stop=True, skip_group_check=True,
        )
        yield
        g1 = sb.tile([128, TILE_N], F32R, tag="g1", name=f"g1_{c}")
        nc.vector.scalar_tensor_tensor(
            g1[:], h1[:], 0.0, g1p[:], op0=ALU.is_gt, op1=ALU.mult
        )
        yield
        g0p = ptmp.tile([128, TILE_N], F32, tag="tmp", name=f"g0p_{c}")
        nc.tensor.matmul(
            g0p[:], wt["Lg0"][:], g1[:],
            start=True, stop=True, skip_group_check=True,
        )
        yield
        g0 = sb.tile([128, TILE_N], F32R, tag="g0", name=f"g0_{c}")
        nc.vector.scalar_tensor_tensor(
            g0[:], h0[:], 0.0, g0p[:], op0=ALU.is_gt, op1=ALU.mult
        )
        yield
        # z0 += blkdiag(P) @ g0  == w0y (x) dy for this step
        nc.tensor.matmul(
            z0[:], wt["LP"][:], g0[:],
            start=False, stop=(s == STEPS - 1), skip_group_check=True,
        )
        yield

    # y = (z0_fin[f*] - W0[f*,0]*x) * inv + y_mean   (inv = 1/W0[f*,1])
    zblk = io.tile([128, TILE_N], F32, tag=f"zb{c}", name=f"zb_{c}")
    nc.scalar.copy(zblk[:], z0[:])
    zf = io.tile([G, TILE_N], F32, tag=f"zf{c}", name=f"zf_{c}")
    for g in range(G):
        r = 32 * g + fstar
        nc.sync.dma_start(out=zf[g:g + 1, :], in_=zblk[r:r + 1, :])
    t1 = io.tile([G, TILE_N], F32, tag=f"t1{c}", name=f"t1_{c}")
    # t1 = x * (W0[f*,0]*inv);  y_mean cancels: z0_init[f*] already
    # includes W0[f*,1]*y_mean, so y = zf*inv - x*cfac exactly.
    nc.vector.tensor_scalar(t1[:], xt[:], cfac, None, ALU.mult)
    yo = io.tile([G, TILE_N], F32, tag=f"yo{c}", name=f"yo_{c}")
    # yo = zf * inv - t1
    nc.vector.scalar_tensor_tensor(yo[:], zf[:], inv, t1[:],
                                   op0=ALU.mult, op1=ALU.subtract)
    nc.sync.dma_start(out=dst, in_=yo[:])
    yield


def build(nt=NT_FULL, fstar=0, inv=1.0, cfac=1.0, reps=1, dyn=None):
    """Build + compile the per-core Bass program for nt packed tiles."""
    nc = bacc.Bacc("TRN2", target_bir_lowering=False, debug=False,
                   num_devices=NCORES)

    ntq = nt // NCHAINS
    dram = {
        "inp0": nc.dram_tensor("inp0", [ntq, NCHAINS, 2 * G, TILE_N], F32R,
                               kind="ExternalInput").ap(),
        "yout": nc.dram_tensor("yout", [ntq, NCHAINS, G, TILE_N], F32,
                               kind="ExternalOutput").ap(),
    }
    wspec = {
        "L0": [2 * G, 128],
        "Lz1": [128, 128], "Lz2": [128, 128],
        "Lg1": [128, 128], "Lg0": [128, 128],
        "LP": [128, 128],
        "b0": [128, 1], "b1": [128, 1], "b2": [128, 1], "nb2": [128, 1],
        "ym": [G, 1],
    }
    wdtype = {k: (F32 if k in ("b0", "b1", "b2", "nb2", "ym") else F32R)
              for k in wspec}
    wdram = {k: nc.dram_tensor(f"w_{k}", sh, wdtype[k],
                               kind="ExternalInput").ap()
             for k, sh in wspec.items()}

    with tile.TileContext(nc) as tc:
        import contextlib
        with contextlib.ExitStack() as ctx:
            wp = ctx.enter_context(tc.tile_pool(name="wp", bufs=1))
            io = ctx.enter_context(tc.tile_pool(name="io", bufs=1))
            sbs = [ctx.enter_context(tc.tile_pool(name=f"sb{c}", bufs=2))
                   for c in range(NCHAINS)]
            if SHARED_TMP:
                pt = ctx.enter_context(
                    tc.tile_pool(name="pt", bufs=TMP_BUFS, space="PSUM"))
                ptmps = [pt] * NCHAINS
            else:
                ptmps = [ctx.enter_context(
                    tc.tile_pool(name=f"pt{c}", bufs=1, space="PSUM"))
                    for c in range(NCHAINS)]
            pz0s = [ctx.enter_context(
                tc.tile_pool(name=f"pz{c}", bufs=1, space="PSUM"))
                for c in range(NCHAINS)]

            wt = {}
            for k, sh in wspec.items():
                wt[k] = wp.tile(sh, wdtype[k], tag=f"w_{k}", name=f"wt_{k}")
                nc.sync.dma_start(out=wt[k][:], in_=wdram[k][:])

            assert nt % NCHAINS == 0

            def emit_quad(tq):
                chains = [
                    _emit_tile_chain(nc, tq, c, dram, wt,
                                     sbs[c], ptmps[c], pz0s[c], io,
                                     fstar, inv, cfac)
                    for c in range(NCHAINS)
                ]
                # phase-offset the chains by ~1/NCHAINS of a step so no
                # engine sees two dependent ops of one chain back-to-back
                prime = 3
                for c, ch in enumerate(chains):
                    for _ in range(c * prime):
                        next(ch)
                alive = list(chains)
                while alive:
                    for ch in list(alive):
                        try:
                            next(ch)
                        except StopIteration:
                            alive.remove(ch)

            use_dyn = DYN if dyn is None else dyn
            if use_dyn:
                def body():
                    with tc.For_i(0, ntq, 1,
                                  hint_engines=(mybir.EngineType.PE,)) as iv:
                        emit_quad(iv)
                if reps > 1:
                    with tc.For_i(0, reps, 1):
                        body()
                else:
                    body()
            else:
                for tq in range(ntq):
                    emit_quad(tq)

    nc.compile()
    return nc


def make_weight_arrays(W0, b0, W1, b1, W2, b2, W3, b3, y_mean):
    """Host-side constant construction (all small)."""
    eye = np.eye(G, dtype=np.float32)
    blk = lambda A: np.kron(eye, A.astype(np.float32))
    w0y = W0[:, 1].astype(np.float32)
    P = (-LR) * np.outer(w0y, w0y)
    ym = np.float32(np.asarray(y_mean).reshape(-1)[0])
    return {
        "w_L0": blk(W0.T),                       # [8, 128]
        "w_Lz1": blk(W1.T),                      # [128, 128]
        "w_Lz2": blk(W2.T),                      # [128, 128]
        "w_Lg1": blk(W2 * W3[0][:, None]),       # [128, 128]
        "w_Lg0": blk(W1),                        # [128, 128]
        "w_LP": blk(P),                          # [128, 128]
        "w_b0": np.tile(b0.astype(np.float32), G)[:, None],
        "w_b1": np.tile(b1.astype(np.float32), G)[:, None],
        "w_b2": np.tile(b2.astype(np.float32), G)[:, None],
        "w_nb2": np.tile(-b2.astype(np.float32), G)[:, None],
        "w_ym": np.full((G, 1), ym, np.float32),
    }


def extraction_consts(W0):
    W0 = np.asarray(W0, np.float32)
    fstar = int(np.argmax(np.abs(W0[:, 1])))
    inv = float(1.0 / W0[fstar, 1])
    cfac = float(W0[fstar, 0] * inv)
    return fstar, inv, cfac


def make_core_inputs(x, y_mean, nt=NT_FULL):
    """Per-core input tiles: [nt, 8, 512] with x on even rows, y_mean on
    odd rows.  Returns a list of NCORES arrays."""
    xs = np.ascontiguousarray(
        np.asarray(x, np.float32).reshape(NCORES, nt, G, TILE_N))
    ym = np.float32(np.asarray(y_mean).reshape(-1)[0])
    maps = []
    for c in range(NCORES):
        inp0 = np.empty((nt, 2 * G, TILE_N), dtype=np.float32)
        inp0[:, 0::2, :] = xs[c]
        inp0[:, 1::2, :] = ym
        maps.append(inp0.reshape(nt // NCHAINS, NCHAINS, 2 * G, TILE_N))
    return maps


_NC_CACHE = {}


def get_nc(nt, fstar, inv, cfac):
    key = (nt, fstar, round(inv, 9), round(cfac, 9))
    if key not in _NC_CACHE:
        _NC_CACHE[key] = build(nt, fstar, inv, cfac)
    return _NC_CACHE[key]


def kernel(x, W0, b0, W1, b1, W2, b2, W3, b3, y_mean):
    x = np.asarray(x, dtype=np.float32)
    fstar, inv, cfac = extraction_consts(W0)
    nc = get_nc(NT_FULL, fstar, inv, cfac)

    warr = make_weight_arrays(
        np.asarray(W0), np.asarray(b0), np.asarray(W1), np.asarray(b1),
        np.asarray(W2), np.asarray(b2), np.asarray(W3), np.asarray(b3),
        np.asarray(y_mean))
    inp0s = make_core_inputs(x, np.asarray(y_mean), NT_FULL)
    in_maps = [{"inp0": inp0s[c], **warr} for c in range(NCORES)]

    res = run_bass_kernel_spmd(nc, in_maps, list(range(NCORES)))
    youts = [res.results[c]["yout"].reshape(BC) for c in range(NCORES)]
    return np.concatenate(youts).reshape(B, 1).astype(np.float32)
